# revision 17
# baseline (speedup 1.0000x reference)
# MoE top-1 routing layer (B=2, S=2048, D=1024, E=8, F=4096) on 8 NeuronCores.
#
# Strategy: expert-parallel. Host computes the (tiny) gate in f64 — the top-2
# logit gap for this problem is >3e-4, far above f32 noise, so argmax matches
# the f32 jax reference exactly. Tokens are dispatched per-expert on the host
# (the "all-to-all"), each core runs the FFN of one expert over its
# capacity-padded token batch, and the host combines results back.
#
# Device layout is feature-major ("transposed tokens"): X^T [D, C], H^T [F, C],
# Y^T [D, C]. Both matmuls then contract along partitions with zero on-device
# transposes, and the biases become per-partition scalars for the ACT engine.
# Matmul operands are bf16 (f32 PSUM accumulation); weights are pre-tiled on
# the host into the exact SBUF tile layout so every DMA is contiguous.

import numpy as np
import ml_dtypes

import concourse.bass as bass
import concourse.mybir as mybir
from concourse.tile import TileContext
from concourse.bass_utils import run_bass_kernel_spmd

B, S, D = 2, 2048, 1024
E, F = 8, 4096
T = B * S
P = 128
KD = D // P    # 8  k-chunks for MM1 (contract over D)
KF = F // P    # 32 k-chunks for MM2 (contract over F)
FT = F // P    # 32 f-tiles (MM1 output partitions)
DT = D // P    # 8  d-tiles (MM2 output partitions)

BF16 = mybir.dt.bfloat16
F32 = mybir.dt.float32
NP_BF16 = ml_dtypes.bfloat16

# The walrus build in this toolchain rejects instructions carrying more than
# one sync-wait ("Too many sync wait commands"), while Tile freely attaches
# several. Spill excess waits onto same-engine nops inserted just before the
# offending instruction — identical semantics, engine streams are in-order.
_MAXW = 1


def _split_excess_waits(nc: bass.Bass):
    for fn in nc.m.functions:
        for blk in fn.blocks:
            lst = blk.instructions
            i = 0
            while i < len(lst):
                inst = lst[i]
                si = getattr(inst, "sync_info", None)
                ow = getattr(si, "on_wait", None) if si is not None else None
                if ow is not None and len(ow) > _MAXW:
                    extra = list(ow[_MAXW:])
                    del si.on_wait[_MAXW:]
                    eng = nc.engines[inst.engine]
                    new_insts = []
                    for j in range(0, len(extra), _MAXW):
                        chunk = extra[j : j + _MAXW]
                        nop = eng.nop(nofuse=True).ins
                        if nop.sync_info is None:
                            nop.sync_info = mybir.SyncInfo(
                                on_wait=chunk, on_update=[]
                            )
                        else:
                            nop.sync_info.on_wait.extend(chunk)
                        new_insts.append(nop)
                    # eng.nop() appended them to the current bb; relocate.
                    cur_lst = nc.cur_bb.bb.instructions
                    for n in new_insts:
                        cur_lst.remove(n)
                    for j, n in enumerate(new_insts):
                        lst.insert(i + j, n)
                    i += len(new_insts)
                i += 1


def _slim_drain_and_barrier(self, tick_clock, wait_clock):
    # Tile's stock epilogue is drain -> barrier -> sem clears -> barrier.
    # The trailing barrier only orders the GpSimd sem clears against the
    # other engines' stream ends; NEFF completion already requires every
    # engine's stream (and the clears with it) to finish, so drop it.
    from concourse.vector_clock import ScopedClock

    nc = self.nc
    drain_inst = nc.sync.drain()
    wait_clock.add_sem_waits(
        drain_inst.ins, ScopedClock({None: tick_clock.global_clock})
    )
    nc.all_engine_barrier()
    assert self.sems is not None
    popped = nc._tile_sem_poison_stack.pop()
    assert popped is self._sem_poison
    nc.clear_and_free_semaphores(list(self.sems.allocated().values()))


TileContext._drain_and_barrier = _slim_drain_and_barrier


def _build_ffn_program(C: int):
    """One expert FFN: yT = W2.T @ gelu(W1.T @ xT + b1) + b2, all feature-major.

    I/O (per core):
      xt [P, KD, C]  bf16   x^T tiled: xt[p, ko, c] = X^T[ko*P + p, c]
      w1 [FT, P, KD, P] bf16: w1[ft, p, ko, fi] = W1[ko*P + p, ft*P + fi]
      w2 [DT, P, KF, P] bf16: w2[dt, p, fo, di] = W2[fo*P + p, dt*P + di]
      b1 [P, FT] f32, b2 [P, DT] f32 (partition-major bias)
      yt [DT, P, C] f32 out: yt[dt, p, c] = Y^T[dt*P + p, c]
    """
    # Column tiles: PSUM caps one matmul at 512 f32 columns. Measured per-MM
    # cost is ~N/2.4GHz + ~10ns fixed (issue floor ~60 cycles), so maximally
    # wide tiles plus one narrow remainder beat a balanced split.
    c_tiles = [(c0, min(512, C - c0)) for c0 in range(0, C, 512)]

    nc = bass.Bass()
    xt_d = nc.declare_dram_parameter("xt", [P, KD, C], BF16, isOutput=False)
    w1_d = nc.declare_dram_parameter("w1", [FT, P, KD, P], BF16, isOutput=False)
    w2_d = nc.declare_dram_parameter("w2", [DT, P, KF, P], BF16, isOutput=False)
    b1_d = nc.declare_dram_parameter("b1", [P, FT], F32, isOutput=False)
    b2_d = nc.declare_dram_parameter("b2", [P, DT], F32, isOutput=False)
    yt_d = nc.declare_dram_parameter("yt", [DT, P, C], F32, isOutput=True)

    gelu = mybir.ActivationFunctionType.Gelu_apprx_tanh

    with TileContext(nc) as tc:
        with (
            tc.tile_pool(name="const", bufs=1) as cpool,
            tc.tile_pool(name="w1p", bufs=6) as w1p,
            tc.tile_pool(name="w2p", bufs=3) as w2p,
            tc.tile_pool(name="outp", bufs=3) as outp,
            tc.tile_pool(name="psum", bufs=4, space="PSUM") as pp,
        ):
            # PE warm-up: the HAM clock gate holds the array at 1.2 GHz until
            # ~3.4 us of sustained activity. Burn dummy matmuls on scratch
            # data while the x/w DMAs are in flight so real work runs warm.
            warm_sb = cpool.tile([P, 512], BF16)
            nc.vector.memset(warm_sb[:], 0.0)
            warm_ps = pp.tile([P, 512], F32, name="warm", tag="ps1")
            for _ in range(11):
                nc.tensor.matmul(warm_ps, lhsT=warm_sb[:, :P], rhs=warm_sb,
                                 start=True, stop=True)

            # First two w1 tiles go out before x so MM1 isn't blocked on a
            # weight landing behind the whole x load on the same queue.
            w1_head = [w1p.tile([P, KD, P], BF16, name="w1_t") for _ in range(2)]
            for ft, t in enumerate(w1_head):
                nc.sync.dma_start(t[:], w1_d[ft])

            # x chunks split across two DMA-issuing engines for parallel queues
            x_sb = cpool.tile([P, KD, C], BF16)
            x_engs = [nc.sync, nc.gpsimd]
            for k in range(KD):
                x_engs[k % 2].dma_start(x_sb[:, k], xt_d[:, k])
            ht_sb = cpool.tile([P, KF, C], BF16)
            b1_sb = cpool.tile([P, FT], F32)
            nc.gpsimd.dma_start(b1_sb[:], b1_d[:])
            b2_sb = cpool.tile([P, DT], F32)
            nc.gpsimd.dma_start(b2_sb[:], b2_d[:])

            # MM1 + bias + gelu: ht[ft] = gelu(W1.T @ xT + b1)[f-tile ft]
            for ft in range(FT):
                if ft < 2:
                    w1_t = w1_head[ft]
                else:
                    w1_t = w1p.tile([P, KD, P], BF16, name="w1_t")
                    nc.sync.dma_start(w1_t[:], w1_d[ft])
                for c0, ncols in c_tiles:
                    ps = pp.tile([P, 512], F32, name="ps1")[:, :ncols]
                    for k in range(KD):
                        nc.tensor.matmul(
                            ps,
                            lhsT=w1_t[:, k],
                            rhs=x_sb[:, k, c0 : c0 + ncols],
                            start=(k == 0),
                            stop=(k == KD - 1),
                        )
                    nc.scalar.activation(
                        ht_sb[:, ft, c0 : c0 + ncols], ps, gelu,
                        bias=b1_sb[:, ft : ft + 1],
                    )

            # MM2 + bias: yt[dt] = (W2.T @ ht + b2)[d-tile dt]
            for dt in range(DT):
                w2_t = w2p.tile([P, KF, P], BF16)
                nc.sync.dma_start(w2_t[:], w2_d[dt])
                for c0, ncols in c_tiles:
                    ps = pp.tile([P, 512], F32, name="ps2")[:, :ncols]
                    for fo in range(KF):
                        nc.tensor.matmul(
                            ps,
                            lhsT=w2_t[:, fo],
                            rhs=ht_sb[:, fo, c0 : c0 + ncols],
                            start=(fo == 0),
                            stop=(fo == KF - 1),
                        )
                    yt_t = outp.tile([P, 512], F32, name="yt_t")[:, :ncols]
                    nc.vector.tensor_tensor(
                        yt_t, ps,
                        b2_sb[:, dt : dt + 1].to_broadcast((P, ncols)),
                        mybir.AluOpType.add,
                    )
                    nc.sync.dma_start(yt_d[dt, :, c0 : c0 + ncols], yt_t)

    _split_excess_waits(nc)
    return nc


_PROGRAM_CACHE: dict[int, bass.Bass] = {}
LAST_RESULT = None  # BassKernelResults of the most recent run (for profiling)


def kernel(x, input_ids, attention_mask, Wg, W1, b1, W2, b2):
    x = np.asarray(x, dtype=np.float32)
    Wg = np.asarray(Wg, dtype=np.float32)
    W1 = np.asarray(W1, dtype=np.float32)
    b1 = np.asarray(b1, dtype=np.float32)
    W2 = np.asarray(W2, dtype=np.float32)
    b2 = np.asarray(b2, dtype=np.float32)

    xt = x.reshape(T, D)

    # ---- host gate (f64; top-2 gaps >> f32 noise so argmax matches jax f32)
    logits = xt.astype(np.float64) @ Wg.astype(np.float64)          # [T, E]
    logits -= logits.max(axis=1, keepdims=True)
    prob = np.exp(logits)
    prob /= prob.sum(axis=1, keepdims=True)                          # [T, E]
    gate = prob.argmax(axis=1)                                       # [T]
    pg = prob[np.arange(T), gate]                                    # [T]
    counts = np.bincount(gate, minlength=E)

    Pm = prob.mean(axis=0)
    frac = counts / counts.sum()
    balance_loss = np.float32(E * np.sum(Pm * frac))
    gate_load = counts.astype(np.int32)

    # ---- dispatch: capacity-pad each expert's tokens, feature-major
    C = max(P, int(-(-int(counts.max()) // 16)) * 16)
    idx = [np.nonzero(gate == e)[0] for e in range(E)]

    xt_bf = xt.astype(NP_BF16)
    in_maps = []
    for e in range(E):
        xe = np.zeros((C, D), dtype=NP_BF16)
        xe[: counts[e]] = xt_bf[idx[e]]
        # [C, D] -> x^T tiled [P, KD, C]
        xh = np.ascontiguousarray(xe.T.reshape(KD, P, C).transpose(1, 0, 2))
        w1h = np.ascontiguousarray(
            W1[e].reshape(KD, P, FT, P).transpose(2, 1, 0, 3)
        ).astype(NP_BF16)
        w2h = np.ascontiguousarray(
            W2[e].reshape(KF, P, DT, P).transpose(2, 1, 0, 3)
        ).astype(NP_BF16)
        b1h = np.ascontiguousarray(b1[e].reshape(FT, P).T)
        b2h = np.ascontiguousarray(b2[e].reshape(DT, P).T)
        in_maps.append({"xt": xh, "w1": w1h, "w2": w2h, "b1": b1h, "b2": b2h})

    if C not in _PROGRAM_CACHE:
        _PROGRAM_CACHE[C] = _build_ffn_program(C)
    nc = _PROGRAM_CACHE[C]

    global LAST_RESULT
    LAST_RESULT = run_bass_kernel_spmd(nc, in_maps, core_ids=list(range(E)))

    # ---- combine: scatter back and apply the gate probability
    out = np.zeros((T, D), dtype=np.float32)
    for e in range(E):
        ye = LAST_RESULT.results[e]["yt"].reshape(D, C).T  # [C, D]
        out[idx[e]] = ye[: counts[e]]
    out *= pg[:, None].astype(np.float32)

    return out.reshape(B, S, D), balance_loss, gate_load


# revision 18
# speedup vs baseline: 1.0159x; 1.0159x over previous
# MoE top-1 routing layer (B=2, S=2048, D=1024, E=8, F=4096) on 8 NeuronCores.
#
# Strategy: expert-parallel. Host computes the (tiny) gate in f64 — the top-2
# logit gap for this problem is >3e-4, far above f32 noise, so argmax matches
# the f32 jax reference exactly. Tokens are dispatched per-expert on the host
# (the "all-to-all"), each core runs the FFN of one expert over its
# capacity-padded token batch, and the host combines results back.
#
# Device layout is feature-major ("transposed tokens"): X^T [D, C], H^T [F, C],
# Y^T [D, C]. Both matmuls then contract along partitions with zero on-device
# transposes, and the biases become per-partition scalars for the ACT engine.
# Matmul operands are bf16 (f32 PSUM accumulation); weights are pre-tiled on
# the host into the exact SBUF tile layout so every DMA is contiguous.

import numpy as np
import ml_dtypes

import concourse.bass as bass
import concourse.mybir as mybir
from concourse.tile import TileContext
from concourse.bass_utils import run_bass_kernel_spmd

B, S, D = 2, 2048, 1024
E, F = 8, 4096
T = B * S
P = 128
KD = D // P    # 8  k-chunks for MM1 (contract over D)
KF = F // P    # 32 k-chunks for MM2 (contract over F)
FT = F // P    # 32 f-tiles (MM1 output partitions)
DT = D // P    # 8  d-tiles (MM2 output partitions)

BF16 = mybir.dt.bfloat16
F32 = mybir.dt.float32
NP_BF16 = ml_dtypes.bfloat16

# The walrus build in this toolchain rejects instructions carrying more than
# one sync-wait ("Too many sync wait commands"), while Tile freely attaches
# several. Spill excess waits onto same-engine nops inserted just before the
# offending instruction — identical semantics, engine streams are in-order.
_MAXW = 1


def _split_excess_waits(nc: bass.Bass):
    for fn in nc.m.functions:
        for blk in fn.blocks:
            lst = blk.instructions
            i = 0
            while i < len(lst):
                inst = lst[i]
                si = getattr(inst, "sync_info", None)
                ow = getattr(si, "on_wait", None) if si is not None else None
                if ow is not None and len(ow) > _MAXW:
                    extra = list(ow[_MAXW:])
                    del si.on_wait[_MAXW:]
                    eng = nc.engines[inst.engine]
                    new_insts = []
                    for j in range(0, len(extra), _MAXW):
                        chunk = extra[j : j + _MAXW]
                        nop = eng.nop(nofuse=True).ins
                        if nop.sync_info is None:
                            nop.sync_info = mybir.SyncInfo(
                                on_wait=chunk, on_update=[]
                            )
                        else:
                            nop.sync_info.on_wait.extend(chunk)
                        new_insts.append(nop)
                    # eng.nop() appended them to the current bb; relocate.
                    cur_lst = nc.cur_bb.bb.instructions
                    for n in new_insts:
                        cur_lst.remove(n)
                    for j, n in enumerate(new_insts):
                        lst.insert(i + j, n)
                    i += len(new_insts)
                i += 1


def _slim_drain_and_barrier(self, tick_clock, wait_clock):
    # Tile's stock epilogue is drain -> barrier -> sem clears -> barrier.
    # The trailing barrier only orders the GpSimd sem clears against the
    # other engines' stream ends; NEFF completion already requires every
    # engine's stream (and the clears with it) to finish, so drop it.
    from concourse.vector_clock import ScopedClock

    nc = self.nc
    drain_inst = nc.sync.drain()
    wait_clock.add_sem_waits(
        drain_inst.ins, ScopedClock({None: tick_clock.global_clock})
    )
    nc.all_engine_barrier()
    assert self.sems is not None
    popped = nc._tile_sem_poison_stack.pop()
    assert popped is self._sem_poison
    nc.clear_and_free_semaphores(list(self.sems.allocated().values()))


TileContext._drain_and_barrier = _slim_drain_and_barrier


def _build_ffn_program(C: int):
    """One expert FFN: yT = W2.T @ gelu(W1.T @ xT + b1) + b2, all feature-major.

    I/O (per core):
      xt [P, KD, C]  bf16   x^T tiled: xt[p, ko, c] = X^T[ko*P + p, c]
      w1 [FT, P, KD, P] bf16: w1[ft, p, ko, fi] = W1[ko*P + p, ft*P + fi]
      w2 [DT, P, KF, P] bf16: w2[dt, p, fo, di] = W2[fo*P + p, dt*P + di]
      b1 [P, FT] f32, b2 [P, DT] f32 (partition-major bias)
      yt [DT, P, C] f32 out: yt[dt, p, c] = Y^T[dt*P + p, c]
    """
    # Column tiles: PSUM caps one matmul at 512 f32 columns. Measured per-MM
    # cost is ~N/2.4GHz + ~10ns fixed (issue floor ~60 cycles), so maximally
    # wide tiles plus one narrow remainder beat a balanced split.
    c_tiles = [(c0, min(512, C - c0)) for c0 in range(0, C, 512)]

    nc = bass.Bass()
    xt_d = nc.declare_dram_parameter("xt", [P, KD, C], BF16, isOutput=False)
    w1_d = nc.declare_dram_parameter("w1", [FT, P, KD, P], BF16, isOutput=False)
    w2_d = nc.declare_dram_parameter("w2", [DT, P, KF, P], BF16, isOutput=False)
    b1_d = nc.declare_dram_parameter("b1", [P, FT], F32, isOutput=False)
    b2_d = nc.declare_dram_parameter("b2", [P, DT], F32, isOutput=False)
    yt_d = nc.declare_dram_parameter("yt", [DT, P, C], F32, isOutput=True)

    gelu = mybir.ActivationFunctionType.Gelu_apprx_tanh

    with TileContext(nc) as tc:
        with (
            tc.tile_pool(name="const", bufs=1) as cpool,
            tc.tile_pool(name="w1p", bufs=6) as w1p,
            tc.tile_pool(name="w2p", bufs=3) as w2p,
            tc.tile_pool(name="outp", bufs=3) as outp,
            tc.tile_pool(name="psum", bufs=4, space="PSUM") as pp,
        ):
            # PE warm-up: the HAM clock gate holds the array at 1.2 GHz until
            # ~3.4 us of sustained activity. Burn dummy matmuls on scratch
            # data while the x/w DMAs are in flight so real work runs warm.
            warm_sb = cpool.tile([P, 512], BF16)
            nc.vector.memset(warm_sb[:], 0.0)
            warm_ps = pp.tile([P, 512], F32, name="warm", tag="ps1")
            for _ in range(15):
                nc.tensor.matmul(warm_ps, lhsT=warm_sb[:, :P], rhs=warm_sb,
                                 start=True, stop=True)

            # First two w1 tiles go out before x so MM1 isn't blocked on a
            # weight landing behind the whole x load on the same queue.
            w1_head = [w1p.tile([P, KD, P], BF16, name="w1_t") for _ in range(2)]
            for ft, t in enumerate(w1_head):
                nc.sync.dma_start(t[:], w1_d[ft])

            # x chunks split across two DMA-issuing engines for parallel queues
            x_sb = cpool.tile([P, KD, C], BF16)
            x_engs = [nc.sync, nc.gpsimd]
            for k in range(KD):
                x_engs[k % 2].dma_start(x_sb[:, k], xt_d[:, k])
            ht_sb = cpool.tile([P, KF, C], BF16)
            b1_sb = cpool.tile([P, FT], F32)
            nc.gpsimd.dma_start(b1_sb[:], b1_d[:])
            b2_sb = cpool.tile([P, DT], F32)
            nc.gpsimd.dma_start(b2_sb[:], b2_d[:])

            # MM1 + bias + gelu: ht[ft] = gelu(W1.T @ xT + b1)[f-tile ft]
            for ft in range(FT):
                if ft < 2:
                    w1_t = w1_head[ft]
                else:
                    w1_t = w1p.tile([P, KD, P], BF16, name="w1_t")
                    nc.sync.dma_start(w1_t[:], w1_d[ft])
                for c0, ncols in c_tiles:
                    ps = pp.tile([P, 512], F32, name="ps1")[:, :ncols]
                    for k in range(KD):
                        nc.tensor.matmul(
                            ps,
                            lhsT=w1_t[:, k],
                            rhs=x_sb[:, k, c0 : c0 + ncols],
                            start=(k == 0),
                            stop=(k == KD - 1),
                        )
                    nc.scalar.activation(
                        ht_sb[:, ft, c0 : c0 + ncols], ps, gelu,
                        bias=b1_sb[:, ft : ft + 1],
                    )

            # MM2 + bias: yt[dt] = (W2.T @ ht + b2)[d-tile dt]
            for dt in range(DT):
                w2_t = w2p.tile([P, KF, P], BF16)
                nc.sync.dma_start(w2_t[:], w2_d[dt])
                for c0, ncols in c_tiles:
                    ps = pp.tile([P, 512], F32, name="ps2")[:, :ncols]
                    for fo in range(KF):
                        nc.tensor.matmul(
                            ps,
                            lhsT=w2_t[:, fo],
                            rhs=ht_sb[:, fo, c0 : c0 + ncols],
                            start=(fo == 0),
                            stop=(fo == KF - 1),
                        )
                    yt_t = outp.tile([P, 512], F32, name="yt_t")[:, :ncols]
                    nc.vector.tensor_tensor(
                        yt_t, ps,
                        b2_sb[:, dt : dt + 1].to_broadcast((P, ncols)),
                        mybir.AluOpType.add,
                    )
                    nc.sync.dma_start(yt_d[dt, :, c0 : c0 + ncols], yt_t)

    _split_excess_waits(nc)
    return nc


_PROGRAM_CACHE: dict[int, bass.Bass] = {}
LAST_RESULT = None  # BassKernelResults of the most recent run (for profiling)


def kernel(x, input_ids, attention_mask, Wg, W1, b1, W2, b2):
    x = np.asarray(x, dtype=np.float32)
    Wg = np.asarray(Wg, dtype=np.float32)
    W1 = np.asarray(W1, dtype=np.float32)
    b1 = np.asarray(b1, dtype=np.float32)
    W2 = np.asarray(W2, dtype=np.float32)
    b2 = np.asarray(b2, dtype=np.float32)

    xt = x.reshape(T, D)

    # ---- host gate (f64; top-2 gaps >> f32 noise so argmax matches jax f32)
    logits = xt.astype(np.float64) @ Wg.astype(np.float64)          # [T, E]
    logits -= logits.max(axis=1, keepdims=True)
    prob = np.exp(logits)
    prob /= prob.sum(axis=1, keepdims=True)                          # [T, E]
    gate = prob.argmax(axis=1)                                       # [T]
    pg = prob[np.arange(T), gate]                                    # [T]
    counts = np.bincount(gate, minlength=E)

    Pm = prob.mean(axis=0)
    frac = counts / counts.sum()
    balance_loss = np.float32(E * np.sum(Pm * frac))
    gate_load = counts.astype(np.int32)

    # ---- dispatch: capacity-pad each expert's tokens, feature-major
    C = max(P, int(-(-int(counts.max()) // 16)) * 16)
    idx = [np.nonzero(gate == e)[0] for e in range(E)]

    xt_bf = xt.astype(NP_BF16)
    in_maps = []
    for e in range(E):
        xe = np.zeros((C, D), dtype=NP_BF16)
        xe[: counts[e]] = xt_bf[idx[e]]
        # [C, D] -> x^T tiled [P, KD, C]
        xh = np.ascontiguousarray(xe.T.reshape(KD, P, C).transpose(1, 0, 2))
        w1h = np.ascontiguousarray(
            W1[e].reshape(KD, P, FT, P).transpose(2, 1, 0, 3)
        ).astype(NP_BF16)
        w2h = np.ascontiguousarray(
            W2[e].reshape(KF, P, DT, P).transpose(2, 1, 0, 3)
        ).astype(NP_BF16)
        b1h = np.ascontiguousarray(b1[e].reshape(FT, P).T)
        b2h = np.ascontiguousarray(b2[e].reshape(DT, P).T)
        in_maps.append({"xt": xh, "w1": w1h, "w2": w2h, "b1": b1h, "b2": b2h})

    if C not in _PROGRAM_CACHE:
        _PROGRAM_CACHE[C] = _build_ffn_program(C)
    nc = _PROGRAM_CACHE[C]

    global LAST_RESULT
    LAST_RESULT = run_bass_kernel_spmd(nc, in_maps, core_ids=list(range(E)))

    # ---- combine: scatter back and apply the gate probability
    out = np.zeros((T, D), dtype=np.float32)
    for e in range(E):
        ye = LAST_RESULT.results[e]["yt"].reshape(D, C).T  # [C, D]
        out[idx[e]] = ye[: counts[e]]
    out *= pg[:, None].astype(np.float32)

    return out.reshape(B, S, D), balance_loss, gate_load


# revision 19
# speedup vs baseline: 1.0253x; 1.0092x over previous
# MoE top-1 routing layer (B=2, S=2048, D=1024, E=8, F=4096) on 8 NeuronCores.
#
# Strategy: expert-parallel. Host computes the (tiny) gate in f64 — the top-2
# logit gap for this problem is >3e-4, far above f32 noise, so argmax matches
# the f32 jax reference exactly. Tokens are dispatched per-expert on the host
# (the "all-to-all"), each core runs the FFN of one expert over its
# capacity-padded token batch, and the host combines results back.
#
# Device layout is feature-major ("transposed tokens"): X^T [D, C], H^T [F, C],
# Y^T [D, C]. Both matmuls then contract along partitions with zero on-device
# transposes, and the biases become per-partition scalars for the ACT engine.
# Matmul operands are bf16 (f32 PSUM accumulation); weights are pre-tiled on
# the host into the exact SBUF tile layout so every DMA is contiguous.

import numpy as np
import ml_dtypes

import concourse.bass as bass
import concourse.mybir as mybir
from concourse.tile import TileContext
from concourse.bass_utils import run_bass_kernel_spmd

B, S, D = 2, 2048, 1024
E, F = 8, 4096
T = B * S
P = 128
KD = D // P    # 8  k-chunks for MM1 (contract over D)
KF = F // P    # 32 k-chunks for MM2 (contract over F)
FT = F // P    # 32 f-tiles (MM1 output partitions)
DT = D // P    # 8  d-tiles (MM2 output partitions)

BF16 = mybir.dt.bfloat16
F32 = mybir.dt.float32
NP_BF16 = ml_dtypes.bfloat16

# The walrus build in this toolchain rejects instructions carrying more than
# one sync-wait ("Too many sync wait commands"), while Tile freely attaches
# several. Spill excess waits onto same-engine nops inserted just before the
# offending instruction — identical semantics, engine streams are in-order.
_MAXW = 1


def _split_excess_waits(nc: bass.Bass):
    for fn in nc.m.functions:
        for blk in fn.blocks:
            lst = blk.instructions
            i = 0
            while i < len(lst):
                inst = lst[i]
                si = getattr(inst, "sync_info", None)
                ow = getattr(si, "on_wait", None) if si is not None else None
                if ow is not None and len(ow) > _MAXW:
                    extra = list(ow[_MAXW:])
                    del si.on_wait[_MAXW:]
                    eng = nc.engines[inst.engine]
                    new_insts = []
                    for j in range(0, len(extra), _MAXW):
                        chunk = extra[j : j + _MAXW]
                        nop = eng.nop(nofuse=True).ins
                        if nop.sync_info is None:
                            nop.sync_info = mybir.SyncInfo(
                                on_wait=chunk, on_update=[]
                            )
                        else:
                            nop.sync_info.on_wait.extend(chunk)
                        new_insts.append(nop)
                    # eng.nop() appended them to the current bb; relocate.
                    cur_lst = nc.cur_bb.bb.instructions
                    for n in new_insts:
                        cur_lst.remove(n)
                    for j, n in enumerate(new_insts):
                        lst.insert(i + j, n)
                    i += len(new_insts)
                i += 1


def _slim_drain_and_barrier(self, tick_clock, wait_clock):
    # Tile's stock epilogue is drain -> barrier -> sem clears -> barrier.
    # The trailing barrier only orders the GpSimd sem clears against the
    # other engines' stream ends; NEFF completion already requires every
    # engine's stream (and the clears with it) to finish, so drop it.
    from concourse.vector_clock import ScopedClock

    nc = self.nc
    drain_inst = nc.sync.drain()
    wait_clock.add_sem_waits(
        drain_inst.ins, ScopedClock({None: tick_clock.global_clock})
    )
    nc.all_engine_barrier()
    assert self.sems is not None
    popped = nc._tile_sem_poison_stack.pop()
    assert popped is self._sem_poison
    nc.clear_and_free_semaphores(list(self.sems.allocated().values()))


TileContext._drain_and_barrier = _slim_drain_and_barrier


def _build_ffn_program(C: int):
    """One expert FFN: yT = W2.T @ gelu(W1.T @ xT + b1) + b2, all feature-major.

    I/O (per core):
      xt [P, KD, C]  bf16   x^T tiled: xt[p, ko, c] = X^T[ko*P + p, c]
      w1 [FT, P, KD, P] bf16: w1[ft, p, ko, fi] = W1[ko*P + p, ft*P + fi]
      w2 [DT, P, KF, P] bf16: w2[dt, p, fo, di] = W2[fo*P + p, dt*P + di]
      b1 [P, FT] f32, b2 [P, DT] f32 (partition-major bias)
      yt [DT, P, C] f32 out: yt[dt, p, c] = Y^T[dt*P + p, c]
    """
    # Column tiles: PSUM caps one matmul at 512 f32 columns. Measured per-MM
    # cost is ~N/2.4GHz + ~10ns fixed (issue floor ~60 cycles), so maximally
    # wide tiles plus one narrow remainder beat a balanced split.
    c_tiles = [(c0, min(512, C - c0)) for c0 in range(0, C, 512)]

    nc = bass.Bass()
    xt_d = nc.declare_dram_parameter("xt", [P, KD, C], BF16, isOutput=False)
    w1_d = nc.declare_dram_parameter("w1", [FT, P, KD, P], BF16, isOutput=False)
    w2_d = nc.declare_dram_parameter("w2", [DT, P, KF, P], BF16, isOutput=False)
    b1_d = nc.declare_dram_parameter("b1", [P, FT], F32, isOutput=False)
    b2_d = nc.declare_dram_parameter("b2", [P, DT], F32, isOutput=False)
    yt_d = nc.declare_dram_parameter("yt", [DT, P, C], F32, isOutput=True)

    gelu = mybir.ActivationFunctionType.Gelu_apprx_tanh

    with TileContext(nc) as tc:
        with (
            tc.tile_pool(name="const", bufs=1) as cpool,
            tc.tile_pool(name="w1p", bufs=6) as w1p,
            tc.tile_pool(name="w2p", bufs=3) as w2p,
            tc.tile_pool(name="outp", bufs=3) as outp,
            tc.tile_pool(name="psum", bufs=4, space="PSUM") as pp,
        ):
            # PE warm-up: the HAM clock gate holds the array at 1.2 GHz until
            # ~3.4 us of sustained activity. Burn dummy matmuls on scratch
            # data while the x/w DMAs are in flight so real work runs warm.
            warm_sb = cpool.tile([P, 512], BF16)
            nc.vector.memset(warm_sb[:], 0.0)
            warm_ps = pp.tile([P, 512], F32, name="warm", tag="ps1")
            for _ in range(15):
                nc.tensor.matmul(warm_ps, lhsT=warm_sb[:, :P], rhs=warm_sb,
                                 start=True, stop=True)

            # First two w1 tiles go out before x so MM1 isn't blocked on a
            # weight landing behind the whole x load on the same queue.
            w1_head = [w1p.tile([P, KD, P], BF16, name="w1_t") for _ in range(2)]
            for ft, t in enumerate(w1_head):
                nc.sync.dma_start(t[:], w1_d[ft])

            # x chunks split across two DMA-issuing engines for parallel queues
            x_sb = cpool.tile([P, KD, C], BF16)
            x_engs = [nc.sync, nc.gpsimd]
            for k in range(KD):
                x_engs[k % 2].dma_start(x_sb[:, k], xt_d[:, k])
            ht_sb = cpool.tile([P, KF, C], BF16)
            b1_sb = cpool.tile([P, FT], F32)
            nc.gpsimd.dma_start(b1_sb[:], b1_d[:])
            b2_sb = cpool.tile([P, DT], F32)
            nc.gpsimd.dma_start(b2_sb[:], b2_d[:])

            # MM1 + bias + gelu: ht[ft] = gelu(W1.T @ xT + b1)[f-tile ft]
            for ft in range(FT):
                if ft < 2:
                    w1_t = w1_head[ft]
                else:
                    w1_t = w1p.tile([P, KD, P], BF16, name="w1_t")
                    nc.sync.dma_start(w1_t[:], w1_d[ft])
                for c0, ncols in c_tiles:
                    ps = pp.tile([P, 512], F32, name="ps1")[:, :ncols]
                    for k in range(KD):
                        nc.tensor.matmul(
                            ps,
                            lhsT=w1_t[:, k],
                            rhs=x_sb[:, k, c0 : c0 + ncols],
                            start=(k == 0),
                            stop=(k == KD - 1),
                        )
                    nc.scalar.activation(
                        ht_sb[:, ft, c0 : c0 + ncols], ps, gelu,
                        bias=b1_sb[:, ft : ft + 1],
                    )

            # MM2 + bias: yt[dt] = (W2.T @ ht + b2)[d-tile dt]
            for dt in range(DT):
                w2_t = w2p.tile([P, KF, P], BF16)
                nc.sync.dma_start(w2_t[:], w2_d[dt])
                for c0, ncols in c_tiles:
                    ps = pp.tile([P, 512], F32, name="ps2")[:, :ncols]
                    for fo in range(KF):
                        nc.tensor.matmul(
                            ps,
                            lhsT=w2_t[:, fo],
                            rhs=ht_sb[:, fo, c0 : c0 + ncols],
                            start=(fo == 0),
                            stop=(fo == KF - 1),
                        )
                    yt_t = outp.tile([P, 512], F32, name="yt_t")[:, :ncols]
                    nc.vector.tensor_tensor(
                        yt_t, ps,
                        b2_sb[:, dt : dt + 1].to_broadcast((P, ncols)),
                        mybir.AluOpType.add,
                    )
                    nc.sync.dma_start(yt_d[dt, :, c0 : c0 + ncols], yt_t)

    _split_excess_waits(nc)
    return nc


_PROGRAM_CACHE: dict[int, bass.Bass] = {}
LAST_RESULT = None  # BassKernelResults of the most recent run (for profiling)


def kernel(x, input_ids, attention_mask, Wg, W1, b1, W2, b2):
    x = np.asarray(x, dtype=np.float32)
    Wg = np.asarray(Wg, dtype=np.float32)
    W1 = np.asarray(W1, dtype=np.float32)
    b1 = np.asarray(b1, dtype=np.float32)
    W2 = np.asarray(W2, dtype=np.float32)
    b2 = np.asarray(b2, dtype=np.float32)

    xt = x.reshape(T, D)

    # ---- host gate (f64; top-2 gaps >> f32 noise so argmax matches jax f32)
    logits = xt.astype(np.float64) @ Wg.astype(np.float64)          # [T, E]
    logits -= logits.max(axis=1, keepdims=True)
    prob = np.exp(logits)
    prob /= prob.sum(axis=1, keepdims=True)                          # [T, E]
    gate = prob.argmax(axis=1)                                       # [T]
    pg = prob[np.arange(T), gate]                                    # [T]
    counts = np.bincount(gate, minlength=E)

    Pm = prob.mean(axis=0)
    frac = counts / counts.sum()
    balance_loss = np.float32(E * np.sum(Pm * frac))
    gate_load = counts.astype(np.int32)

    # ---- dispatch: capacity-pad each expert's tokens, feature-major
    C = max(P, int(-(-int(counts.max()) // 16)) * 16)
    idx = [np.nonzero(gate == e)[0] for e in range(E)]

    xt_bf = xt.astype(NP_BF16)
    in_maps = []
    for e in range(E):
        xe = np.zeros((C, D), dtype=NP_BF16)
        xe[: counts[e]] = xt_bf[idx[e]]
        # [C, D] -> x^T tiled [P, KD, C]
        xh = np.ascontiguousarray(xe.T.reshape(KD, P, C).transpose(1, 0, 2))
        w1h = np.ascontiguousarray(
            W1[e].reshape(KD, P, FT, P).transpose(2, 1, 0, 3)
        ).astype(NP_BF16)
        w2h = np.ascontiguousarray(
            W2[e].reshape(KF, P, DT, P).transpose(2, 1, 0, 3)
        ).astype(NP_BF16)
        b1h = np.ascontiguousarray(b1[e].reshape(FT, P).T)
        b2h = np.ascontiguousarray(b2[e].reshape(DT, P).T)
        in_maps.append({"xt": xh, "w1": w1h, "w2": w2h, "b1": b1h, "b2": b2h})

    if C not in _PROGRAM_CACHE:
        _PROGRAM_CACHE[C] = _build_ffn_program(C)
    nc = _PROGRAM_CACHE[C]

    global LAST_RESULT
    try:
        LAST_RESULT = run_bass_kernel_spmd(nc, in_maps, core_ids=list(range(E)))
    except Exception:
        # The axon runtime occasionally reports a transient
        # NRT_EXEC_UNIT_UNRECOVERABLE; a plain retry succeeds.
        LAST_RESULT = run_bass_kernel_spmd(nc, in_maps, core_ids=list(range(E)))

    # ---- combine: scatter back and apply the gate probability
    out = np.zeros((T, D), dtype=np.float32)
    for e in range(E):
        ye = LAST_RESULT.results[e]["yt"].reshape(D, C).T  # [C, D]
        out[idx[e]] = ye[: counts[e]]
    out *= pg[:, None].astype(np.float32)

    return out.reshape(B, S, D), balance_loss, gate_load
